# revision 9
# baseline (speedup 1.0000x reference)
"""Trainium2 Bass kernel for nn_Aggregator (GNN message passing).

Computation per (b, e):
  scores[k] = <side[b], rel[b,e,k,:]>          (contract over D=64)
  attn      = softmax_k(scores)
  agg[d]    = sum_k attn[k] * nbr[b,e,k,d]     (contract over K=32)
  out       = relu(cat(self[b,e], agg) @ W + bias)

Sharding: data-parallel over the leading batch dim B=1024 across 8 cores
(128 batches/core). Weights replicated.

Per-core mapping (fully unrolled over 32 "bgroups" of 4 batches):
  - rel tile   [128p=(4b,32e), (32k,64d)]  -- sequential DMA
  - scores     via fused DVE tensor_tensor_reduce (one op per k)
  - softmax    over k on the free axis (reduce/recip/scale: all native)
  - attn       stream-transposed per 32x32 block -> [(4b,32k), 32e]
  - agg        PE matmuls: lhsT = block-diag attn [128,4], rhs = nbr tile
               [128p=(4b,32k), 64d] -> out rows (e, b-local) in PSUM
  - linear     PE transpose of cat(self, agg) then matmul with W
"""

import os
import numpy as np

B, E, K, D = 1024, 32, 32, 64
NCORES = 8
BC = B // NCORES  # 128 batches per core
NJ = BC // 4      # 32 bgroups of 4 batches

_CACHE = {}


def _build_nc():
    from contextlib import ExitStack

    import concourse.bass as bass
    import concourse.bacc as bacc
    import concourse.tile as tile
    from concourse import mybir

    f32 = mybir.dt.float32
    Alu = mybir.AluOpType
    Act = mybir.ActivationFunctionType

    # Bacc (not raw Bass): its finalize() legalizes sync waits -- TRN2 allows
    # at most 1 wait per instruction; excess waits split into EventSemaphores.
    nc = bacc.Bacc()

    rel_h = nc.declare_dram_parameter("rel", [BC, E, K, D], f32, isOutput=False)
    nbr_h = nc.declare_dram_parameter("nbr", [BC, E, K, D], f32, isOutput=False)
    selfv_h = nc.declare_dram_parameter("selfv", [BC, E, D], f32, isOutput=False)
    side_h = nc.declare_dram_parameter("side", [BC, D], f32, isOutput=False)
    w_h = nc.declare_dram_parameter("wmat", [2 * D, D], f32, isOutput=False)
    b_h = nc.declare_dram_parameter("bvec", [1, D], f32, isOutput=False)
    m01_h = nc.declare_dram_parameter("m01", [128, 4], f32, isOutput=False)
    iden_h = nc.declare_dram_parameter("iden", [128, 128], f32, isOutput=False)
    out_h = nc.declare_dram_parameter("out", [BC, E, D], f32, isOutput=True)

    rel_ap = rel_h[:]
    nbr_ap = nbr_h[:]
    selfv_ap = selfv_h[:]
    side_ap = side_h[:]
    out_ap = out_h[:]

    with tile.TileContext(nc) as tc, ExitStack() as ctx:
        consts = ctx.enter_context(tc.tile_pool(name="consts", bufs=1))
        bigrel = ctx.enter_context(tc.tile_pool(name="bigrel", bufs=3))
        bignbr = ctx.enter_context(tc.tile_pool(name="bignbr", bufs=3))
        work = ctx.enter_context(tc.tile_pool(name="work", bufs=3))
        scratch = ctx.enter_context(tc.tile_pool(name="scratch", bufs=2))
        ps_agg = ctx.enter_context(tc.tile_pool(name="ps_agg", bufs=2, space="PSUM"))
        ps_xt = ctx.enter_context(tc.tile_pool(name="ps_xt", bufs=2, space="PSUM"))
        ps_lin = ctx.enter_context(tc.tile_pool(name="ps_lin", bufs=2, space="PSUM"))

        w_sb = consts.tile([128, D], f32)
        nc.sync.dma_start(out=w_sb, in_=w_h[:])
        m01_sb = consts.tile([128, 4], f32)
        nc.sync.dma_start(out=m01_sb, in_=m01_h[:])
        iden_sb = consts.tile([128, 128], f32)
        nc.sync.dma_start(out=iden_sb, in_=iden_h[:])
        btile = consts.tile([128, D], f32)
        nc.sync.dma_start(
            out=btile,
            in_=bass.AP(tensor=b_h[:].tensor, offset=0, ap=[[0, 128], [1, D]]),
        )
        # side_all[p=(bg,e), j, d] = side[4j + bg, d]; loaded once, one DMA
        # per bg-row block (3-dim AP limit).
        side_all = consts.tile([128, NJ, D], f32)
        for bg in range(4):
            nc.sync.dma_start(
                out=side_all[32 * bg : 32 * bg + 32],
                in_=bass.AP(
                    tensor=side_h[:].tensor,
                    offset=bg * D,
                    ap=[[0, 32], [4 * D, NJ], [1, D]],
                ),
            )

        for j in range(NJ):
            # --- loads ---
            rel_sb = bigrel.tile([128, K, D], f32, tag="rel")
            nc.sync.dma_start(out=rel_sb, in_=rel_ap[4 * j : 4 * j + 4])
            nbr_sb = bignbr.tile([128, E, D], f32, tag="nbr")
            for i in range(4):
                nc.sync.dma_start(
                    out=nbr_sb[32 * i : 32 * i + 32],
                    in_=nbr_ap[4 * j + i].transpose([1, 0, 2]),
                )
            side4 = side_all[:, j, :]
            # side broadcast over k: [128, K, D] view with a 0-step k dim
            side4_bk = bass.AP(
                tensor=side4.tensor,
                offset=side4.offset,
                ap=[side4.ap[0], [0, K], side4.ap[-1]],
            )

            # --- scores[p=(b,e), k] = <rel[p,k,:], side[b,:]> ---
            prod = scratch.tile([128, K, D], f32, tag="prod")
            nc.vector.tensor_mul(out=prod, in0=rel_sb, in1=side4_bk)
            scores = work.tile([128, K], f32, tag="scores")
            nc.vector.tensor_reduce(
                out=scores, in_=prod, axis=mybir.AxisListType.X, op=Alu.add
            )

            # --- softmax over k (free axis; no max-subtraction needed:
            #     |scores| <~ 6*sqrt(64) stays well inside f32 exp range) ---
            escores = work.tile([128, K], f32, tag="escores")
            nc.scalar.activation(out=escores, in_=scores, func=Act.Exp)
            sums = work.tile([128, 1], f32, tag="sums")
            nc.vector.tensor_reduce(
                out=sums, in_=escores, axis=mybir.AxisListType.X, op=Alu.add
            )
            rsums = work.tile([128, 1], f32, tag="rsums")
            nc.vector.reciprocal(out=rsums, in_=sums)
            attn = work.tile([128, K], f32, tag="attn")
            nc.vector.tensor_scalar_mul(out=attn, in0=escores, scalar1=rsums)

            # --- rearrange attn to [(4b,32k), 32e] (per-32x32-block transpose) ---
            attn_t = work.tile([128, K], f32, tag="attn_t")
            nc.vector.transpose(out=attn_t, in_=attn)

            # --- block-diagonal attn: attn_bd[p, e, i] = attn_t[p, e] * (p//32 == i) ---
            attn_bd = work.tile([128, E, 4], f32, tag="attn_bd")
            for i in range(4):
                nc.vector.tensor_scalar_mul(
                    out=attn_bd[:, :, i], in0=attn_t, scalar1=m01_sb[:, i : i + 1]
                )

            # --- aggT[d, (e,i)] via PE: out[:, 4e:4e+4] = nbr[:,e,:].T @ attn_bd[:,e,:]
            #     (feature-major; PE psum writes must start at partition 0/32/64) ---
            agg_ps = ps_agg.tile([D, 4 * E], f32, tag="agg")
            for e in range(E):
                nc.tensor.matmul(
                    out=agg_ps[:, 4 * e : 4 * e + 4],
                    lhsT=nbr_sb[:, e, :],
                    rhs=attn_bd[:, e, :],
                    start=True,
                    stop=True,
                )

            # --- selfT[d, (e,i)] via PE transpose of self rows ---
            self_sb = work.tile([128, D], f32, tag="self_sb")
            nc.sync.dma_start(
                out=self_sb, in_=selfv_ap[4 * j : 4 * j + 4].transpose([1, 0, 2])
            )
            st_ps = ps_xt.tile([D, 128], f32, tag="st")
            nc.tensor.transpose(out=st_ps, in_=self_sb, identity=iden_sb)

            # --- X^T = [selfT; aggT] (feature-major cat) ---
            xt_sb = work.tile([128, 128], f32, tag="xt_sb")
            nc.scalar.copy(out=xt_sb[0:D, :], in_=st_ps)
            nc.scalar.copy(out=xt_sb[D : 2 * D, :], in_=agg_ps)
            lin_ps = ps_lin.tile([128, D], f32, tag="lin")
            nc.tensor.matmul(
                out=lin_ps, lhsT=xt_sb, rhs=w_sb, start=True, stop=True
            )
            outb = work.tile([128, D], f32, tag="outb")
            nc.vector.tensor_add(out=outb, in0=lin_ps, in1=btile)
            nc.vector.tensor_scalar_max(out=outb, in0=outb, scalar1=0.0)
            nc.sync.dma_start(
                out=out_ap[4 * j : 4 * j + 4].transpose([1, 0, 2]), in_=outb
            )

    nc.finalize()
    return nc


def _get_nc():
    if "nc" not in _CACHE:
        _CACHE["nc"] = _build_nc()
    return _CACHE["nc"]


def _make_in_maps(self_vectors, neighbor_vectors, neighbor_relations, side_embeddings, W, b):
    m01 = np.zeros((128, 4), dtype=np.float32)
    for i in range(4):
        m01[32 * i : 32 * i + 32, i] = 1.0
    iden = np.eye(128, dtype=np.float32)
    in_maps = []
    for c in range(NCORES):
        sl = slice(c * BC, (c + 1) * BC)
        in_maps.append(
            {
                "rel": np.ascontiguousarray(neighbor_relations[sl], dtype=np.float32),
                "nbr": np.ascontiguousarray(neighbor_vectors[sl], dtype=np.float32),
                "selfv": np.ascontiguousarray(self_vectors[sl], dtype=np.float32),
                "side": np.ascontiguousarray(side_embeddings[sl], dtype=np.float32),
                "wmat": np.ascontiguousarray(W, dtype=np.float32),
                "bvec": np.ascontiguousarray(b, dtype=np.float32).reshape(1, D),
                "m01": m01,
                "iden": iden,
            }
        )
    return in_maps


def kernel(self_vectors, neighbor_vectors, neighbor_relations, side_embeddings, W, b,
           _trace=False, _tmpdir=None):
    from concourse import bass_utils

    nc = _get_nc()
    in_maps = _make_in_maps(
        self_vectors, neighbor_vectors, neighbor_relations, side_embeddings, W, b
    )
    res = bass_utils.run_bass_kernel_spmd(
        nc, in_maps, list(range(NCORES)), trace=_trace, tmpdir=_tmpdir
    )
    _CACHE["last_results"] = res
    out = np.concatenate([res.results[c]["out"] for c in range(NCORES)], axis=0)
    return out


# revision 11
# speedup vs baseline: 1.1989x; 1.1989x over previous
"""Trainium2 Bass kernel for nn_Aggregator (GNN message passing).

Computation per (b, e):
  scores[k] = <side[b], rel[b,e,k,:]>          (contract over D=64)
  attn      = softmax_k(scores)
  agg[d]    = sum_k attn[k] * nbr[b,e,k,d]     (contract over K=32)
  out       = relu(cat(self[b,e], agg) @ W + bias)

Sharding: data-parallel over the leading batch dim B=1024 across 8 cores
(128 batches/core). Weights replicated.

Per-core mapping (fully unrolled over 32 "bgroups" of 4 batches), with
ALL tiles on (4b x 32e) partitions so softmax and both contractions are
per-partition free-axis ops on the DVE; the PE only does the final
Linear (transpose + matmul). neighbor_vectors is fed host-permuted to
[b, e, d, k] so the K-contraction is an innermost-axis reduce and every
DMA is fully sequential in DRAM.
"""

import numpy as np

B, E, K, D = 1024, 32, 32, 64
NCORES = 8
BC = B // NCORES  # 128 batches per core
NJ = BC // 4      # 32 bgroups of 4 batches

_CACHE = {}


def _build_nc():
    from contextlib import ExitStack

    import concourse.bass as bass
    import concourse.bacc as bacc
    import concourse.tile as tile
    from concourse import mybir

    f32 = mybir.dt.float32
    Alu = mybir.AluOpType
    Act = mybir.ActivationFunctionType

    # Bacc (not raw Bass): its finalize() legalizes sync waits -- TRN2 allows
    # at most 1 wait per instruction; excess waits split into EventSemaphores.
    nc = bacc.Bacc()

    rel_h = nc.declare_dram_parameter("rel", [BC, E, K, D], f32, isOutput=False)
    nbrt_h = nc.declare_dram_parameter("nbrt", [BC, E, D, K], f32, isOutput=False)
    selfv_h = nc.declare_dram_parameter("selfv", [BC, E, D], f32, isOutput=False)
    side_h = nc.declare_dram_parameter("side", [BC, D], f32, isOutput=False)
    w_h = nc.declare_dram_parameter("wmat", [2 * D, D], f32, isOutput=False)
    b_h = nc.declare_dram_parameter("bvec", [1, D], f32, isOutput=False)
    iden_h = nc.declare_dram_parameter("iden", [128, 128], f32, isOutput=False)
    out_h = nc.declare_dram_parameter("out", [BC, E, D], f32, isOutput=True)

    rel_ap = rel_h[:]
    nbrt_ap = nbrt_h[:]
    selfv_ap = selfv_h[:]
    out_ap = out_h[:]

    with tile.TileContext(nc) as tc, ExitStack() as ctx:
        consts = ctx.enter_context(tc.tile_pool(name="consts", bufs=1))
        bigrel = ctx.enter_context(tc.tile_pool(name="bigrel", bufs=3))
        bignbr = ctx.enter_context(tc.tile_pool(name="bignbr", bufs=3))
        prods = ctx.enter_context(tc.tile_pool(name="prods", bufs=2))
        work = ctx.enter_context(tc.tile_pool(name="work", bufs=3))
        ps_xt = ctx.enter_context(tc.tile_pool(name="ps_xt", bufs=2, space="PSUM"))
        ps_lin = ctx.enter_context(tc.tile_pool(name="ps_lin", bufs=2, space="PSUM"))

        w_sb = consts.tile([128, D], f32)
        nc.sync.dma_start(out=w_sb, in_=w_h[:])
        iden_sb = consts.tile([128, 128], f32)
        nc.sync.dma_start(out=iden_sb, in_=iden_h[:])
        btile = consts.tile([128, D], f32)
        nc.sync.dma_start(
            out=btile,
            in_=bass.AP(tensor=b_h[:].tensor, offset=0, ap=[[0, 128], [1, D]]),
        )
        # side_all[p=(bg,e), j, d] = side[4j + bg, d]; loaded once, one DMA
        # per bg-row block (3-dim AP limit), then copied so the per-j reads
        # depend on a single producer.
        side_all = consts.tile([128, NJ, D], f32)
        for bg in range(4):
            nc.sync.dma_start(
                out=side_all[32 * bg : 32 * bg + 32],
                in_=bass.AP(
                    tensor=side_h[:].tensor,
                    offset=bg * D,
                    ap=[[0, 32], [4 * D, NJ], [1, D]],
                ),
            )

        for j in range(NJ):
            # --- loads: fully sequential DRAM reads, (4b,32e) partitions ---
            rel_sb = bigrel.tile([128, K, D], f32, tag="rel")
            nc.sync.dma_start(out=rel_sb, in_=rel_ap[4 * j : 4 * j + 4])
            nbrt_sb = bignbr.tile([128, D, K], f32, tag="nbr")
            nc.sync.dma_start(out=nbrt_sb, in_=nbrt_ap[4 * j : 4 * j + 4])

            side4 = side_all[:, j, :]
            # side broadcast over k: [128, K, D] view with a 0-step k dim
            side4_bk = bass.AP(
                tensor=side4.tensor,
                offset=side4.offset,
                ap=[side4.ap[0], [0, K], side4.ap[-1]],
            )

            # --- scores[p, k] = sum_d rel[p,k,d] * side[b(p),d] ---
            prod = prods.tile([128, K, D], f32, tag="prod")
            nc.vector.tensor_mul(out=prod, in0=rel_sb, in1=side4_bk)
            scores = work.tile([128, K], f32, tag="scores")
            nc.vector.tensor_reduce(
                out=scores, in_=prod, axis=mybir.AxisListType.X, op=Alu.add
            )

            # --- softmax over k (free axis; no max-subtraction needed:
            #     |scores| <~ 6*sqrt(64) stays well inside f32 exp range) ---
            escores = work.tile([128, K], f32, tag="escores")
            nc.scalar.activation(out=escores, in_=scores, func=Act.Exp)
            sums = work.tile([128, 1], f32, tag="sums")
            nc.vector.tensor_reduce(
                out=sums, in_=escores, axis=mybir.AxisListType.X, op=Alu.add
            )
            rsums = work.tile([128, 1], f32, tag="rsums")
            nc.vector.reciprocal(out=rsums, in_=sums)
            attn = work.tile([128, K], f32, tag="attn")
            nc.vector.tensor_scalar_mul(out=attn, in0=escores, scalar1=rsums)

            # --- agg[p, d] = sum_k attn[p,k] * nbrt[p,d,k] ---
            attn_bdk = bass.AP(
                tensor=attn.tensor,
                offset=attn.offset,
                ap=[attn.ap[0], [0, D], attn.ap[-1]],
            )
            prod2 = prods.tile([128, D, K], f32, tag="prod2")
            nc.vector.tensor_mul(out=prod2, in0=nbrt_sb, in1=attn_bdk)
            # --- X = cat(self, agg), rows (b, e) ---
            xcat = work.tile([128, 2 * D], f32, tag="xcat")
            nc.sync.dma_start(out=xcat[:, 0:D], in_=selfv_ap[4 * j : 4 * j + 4])
            nc.vector.tensor_reduce(
                out=xcat[:, D : 2 * D], in_=prod2, axis=mybir.AxisListType.X, op=Alu.add
            )

            # --- linear: out = relu(X @ W + b) via X^T on PE ---
            xt_ps = ps_xt.tile([128, 2 * D], f32, tag="xt")
            nc.tensor.transpose(out=xt_ps, in_=xcat, identity=iden_sb)
            xt_sb = work.tile([128, 2 * D], f32, tag="xt_sb")
            nc.scalar.copy(out=xt_sb, in_=xt_ps)
            lin_ps = ps_lin.tile([128, D], f32, tag="lin")
            nc.tensor.matmul(
                out=lin_ps, lhsT=xt_sb, rhs=w_sb, start=True, stop=True
            )
            outb = work.tile([128, D], f32, tag="outb")
            nc.vector.tensor_add(out=outb, in0=lin_ps, in1=btile)
            nc.vector.tensor_scalar_max(out=outb, in0=outb, scalar1=0.0)
            nc.sync.dma_start(out=out_ap[4 * j : 4 * j + 4], in_=outb)

    nc.finalize()
    return nc


def _get_nc():
    if "nc" not in _CACHE:
        _CACHE["nc"] = _build_nc()
    return _CACHE["nc"]


def _make_in_maps(self_vectors, neighbor_vectors, neighbor_relations, side_embeddings, W, b):
    iden = np.eye(128, dtype=np.float32)
    nbrt = np.ascontiguousarray(
        np.asarray(neighbor_vectors, dtype=np.float32).transpose(0, 1, 3, 2)
    )
    in_maps = []
    for c in range(NCORES):
        sl = slice(c * BC, (c + 1) * BC)
        in_maps.append(
            {
                "rel": np.ascontiguousarray(neighbor_relations[sl], dtype=np.float32),
                "nbrt": nbrt[sl],
                "selfv": np.ascontiguousarray(self_vectors[sl], dtype=np.float32),
                "side": np.ascontiguousarray(side_embeddings[sl], dtype=np.float32),
                "wmat": np.ascontiguousarray(W, dtype=np.float32),
                "bvec": np.ascontiguousarray(b, dtype=np.float32).reshape(1, D),
                "iden": iden,
            }
        )
    return in_maps


def kernel(self_vectors, neighbor_vectors, neighbor_relations, side_embeddings, W, b,
           _trace=False, _tmpdir=None):
    from concourse import bass_utils

    nc = _get_nc()
    in_maps = _make_in_maps(
        self_vectors, neighbor_vectors, neighbor_relations, side_embeddings, W, b
    )
    res = bass_utils.run_bass_kernel_spmd(
        nc, in_maps, list(range(NCORES)), trace=_trace, tmpdir=_tmpdir
    )
    _CACHE["last_results"] = res
    out = np.concatenate([res.results[c]["out"] for c in range(NCORES)], axis=0)
    return out


# revision 17
# speedup vs baseline: 1.4189x; 1.1835x over previous
"""Trainium2 Bass kernel for nn_Aggregator (GNN message passing).

Computation per (b, e):
  scores[k] = <side[b], rel[b,e,k,:]>          (contract over D=64)
  attn      = softmax_k(scores)
  agg[d]    = sum_k attn[k] * nbr[b,e,k,d]     (contract over K=32)
  out       = relu(cat(self[b,e], agg) @ W + bias)

Sharding: data-parallel over the leading batch dim B=1024 across 8 cores
(128 batches/core). Weights replicated.

Per-core mapping (fully unrolled over 32 "bgroups" of 4 batches), with
ALL tiles on (4b x 32e) partitions so softmax and both contractions are
per-partition free-axis ops on the DVE; the PE only does the final
Linear (transpose + matmul). neighbor_vectors is fed host-permuted to
[b, e, d, k] so the K-contraction is an innermost-axis reduce and every
DMA is fully sequential in DRAM.
"""

import numpy as np

B, E, K, D = 1024, 32, 32, 64
NCORES = 8
BC = B // NCORES  # 128 batches per core
NJ = BC // 4      # 32 bgroups of 4 batches

_CACHE = {}


def _build_nc():
    from contextlib import ExitStack

    import concourse.bass as bass
    import concourse.bacc as bacc
    import concourse.tile as tile
    from concourse import mybir

    f32 = mybir.dt.float32
    Alu = mybir.AluOpType
    Act = mybir.ActivationFunctionType

    # Bacc (not raw Bass): its finalize() legalizes sync waits -- TRN2 allows
    # at most 1 wait per instruction; excess waits split into EventSemaphores.
    nc = bacc.Bacc()

    # rel and nbr^T packed along the last axis: rn[b,e,0:2048] = rel[b,e]
    # (k-major), rn[b,e,2048:4096] = nbr[b,e]^T (d-major) -- one DMA per j.
    rn_h = nc.declare_dram_parameter("rn", [BC, E, 2 * K * D], f32, isOutput=False)
    selfv_h = nc.declare_dram_parameter("selfv", [BC, E, D], f32, isOutput=False)
    side_h = nc.declare_dram_parameter("side", [BC, D], f32, isOutput=False)
    w_h = nc.declare_dram_parameter("wmat", [2 * D, D], f32, isOutput=False)
    b_h = nc.declare_dram_parameter("bvec", [1, D], f32, isOutput=False)
    iden_h = nc.declare_dram_parameter("iden", [128, 128], f32, isOutput=False)
    out_h = nc.declare_dram_parameter("out", [BC, E, D], f32, isOutput=True)

    rn_ap = rn_h[:]
    selfv_ap = selfv_h[:]
    out_ap = out_h[:]

    with tile.TileContext(nc) as tc, ExitStack() as ctx:
        consts = ctx.enter_context(tc.tile_pool(name="consts", bufs=1))
        bigrn = ctx.enter_context(tc.tile_pool(name="bigrn", bufs=3))
        prods = ctx.enter_context(tc.tile_pool(name="prods", bufs=2))
        work = ctx.enter_context(tc.tile_pool(name="work", bufs=3))
        # scores intermediate lives in PSUM: DVE's PSUM port set is separate
        # from its SBUF ports, dodging SBUF read/write contention
        ps_prod = ctx.enter_context(tc.tile_pool(name="ps_prod", bufs=1, space="PSUM"))
        ps_xt = ctx.enter_context(tc.tile_pool(name="ps_xt", bufs=2, space="PSUM"))
        ps_lin = ctx.enter_context(tc.tile_pool(name="ps_lin", bufs=2, space="PSUM"))

        w_sb = consts.tile([128, D], f32)
        nc.sync.dma_start(out=w_sb, in_=w_h[:])
        iden_sb = consts.tile([128, 128], f32)
        nc.sync.dma_start(out=iden_sb, in_=iden_h[:])
        btile = consts.tile([128, D], f32)
        nc.sync.dma_start(
            out=btile,
            in_=bass.AP(tensor=b_h[:].tensor, offset=0, ap=[[0, 128], [1, D]]),
        )
        # side_all[p=(bg,e), j, d] = side[4j + bg, d]; loaded once, one DMA
        # per bg-row block (3-dim AP limit), then copied so the per-j reads
        # depend on a single producer.
        side_all = consts.tile([128, NJ, D], f32)
        for bg in range(4):
            nc.sync.dma_start(
                out=side_all[32 * bg : 32 * bg + 32],
                in_=bass.AP(
                    tensor=side_h[:].tensor,
                    offset=bg * D,
                    ap=[[0, 32], [4 * D, NJ], [1, D]],
                ),
            )

        for j in range(NJ):
            # --- load: one fully sequential DMA, (4b,32e) partitions ---
            rn_sb = bigrn.tile([128, 2 * K * D], f32, tag="rn")
            nc.sync.dma_start(out=rn_sb, in_=rn_ap[4 * j : 4 * j + 4])
            rel_sb = rn_sb[:, 0 : K * D].rearrange("p (k d) -> p k d", k=K)
            nbrt_sb = rn_sb[:, K * D : 2 * K * D].rearrange("p (d k) -> p d k", d=D)

            side4 = side_all[:, j, :]
            # side broadcast over k: [128, K, D] view with a 0-step k dim
            side4_bk = bass.AP(
                tensor=side4.tensor,
                offset=side4.offset,
                ap=[side4.ap[0], [0, K], side4.ap[-1]],
            )

            # --- scores[p, k] = sum_d rel[p,k,d] * side[b(p),d] ---
            prod = ps_prod.tile([128, K, D], f32, tag="prod")
            nc.vector.tensor_mul(out=prod, in0=rel_sb, in1=side4_bk)
            scores = work.tile([128, K], f32, tag="scores")
            nc.vector.tensor_reduce(
                out=scores, in_=prod, axis=mybir.AxisListType.X, op=Alu.add
            )

            # --- softmax over k (free axis; no max-subtraction needed:
            #     |scores| <~ 6*sqrt(64) stays well inside f32 exp range) ---
            escores = work.tile([128, K], f32, tag="escores")
            nc.scalar.activation(out=escores, in_=scores, func=Act.Exp)
            sums = work.tile([128, 1], f32, tag="sums")
            nc.vector.tensor_reduce(
                out=sums, in_=escores, axis=mybir.AxisListType.X, op=Alu.add
            )
            rsums = work.tile([128, 1], f32, tag="rsums")
            nc.vector.reciprocal(out=rsums, in_=sums)
            attn = work.tile([128, K], f32, tag="attn")
            nc.vector.tensor_scalar_mul(out=attn, in0=escores, scalar1=rsums)

            # --- agg[p, d] = sum_k attn[p,k] * nbrt[p,d,k] ---
            attn_bdk = bass.AP(
                tensor=attn.tensor,
                offset=attn.offset,
                ap=[attn.ap[0], [0, D], attn.ap[-1]],
            )
            prod2 = prods.tile([128, D, K], f32, tag="prod2")
            nc.vector.tensor_mul(out=prod2, in0=nbrt_sb, in1=attn_bdk)
            # --- X = cat(self, agg), rows (b, e) ---
            xcat = work.tile([128, 2 * D], f32, tag="xcat")
            nc.sync.dma_start(out=xcat[:, 0:D], in_=selfv_ap[4 * j : 4 * j + 4])
            nc.vector.tensor_reduce(
                out=xcat[:, D : 2 * D], in_=prod2, axis=mybir.AxisListType.X, op=Alu.add
            )

            # --- linear: out = relu(X @ W + b) via X^T on PE ---
            xt_ps = ps_xt.tile([128, 2 * D], f32, tag="xt")
            nc.tensor.transpose(out=xt_ps, in_=xcat, identity=iden_sb)
            xt_sb = work.tile([128, 2 * D], f32, tag="xt_sb")
            nc.scalar.copy(out=xt_sb, in_=xt_ps)
            lin_ps = ps_lin.tile([128, D], f32, tag="lin")
            nc.tensor.matmul(
                out=lin_ps, lhsT=xt_sb, rhs=w_sb, start=True, stop=True
            )
            tmpb = work.tile([128, D], f32, tag="tmpb")
            nc.vector.tensor_add(out=tmpb, in0=lin_ps, in1=btile)
            outb = work.tile([128, D], f32, tag="outb")
            nc.scalar.activation(out=outb, in_=tmpb, func=Act.Relu)
            nc.sync.dma_start(out=out_ap[4 * j : 4 * j + 4], in_=outb)

    nc.finalize()
    return nc


def _get_nc():
    if "nc" not in _CACHE:
        _CACHE["nc"] = _build_nc()
    return _CACHE["nc"]


def _make_in_maps(self_vectors, neighbor_vectors, neighbor_relations, side_embeddings, W, b):
    iden = np.eye(128, dtype=np.float32)
    rel = np.asarray(neighbor_relations, dtype=np.float32).reshape(B, E, K * D)
    nbrt = (
        np.asarray(neighbor_vectors, dtype=np.float32)
        .transpose(0, 1, 3, 2)
        .reshape(B, E, D * K)
    )
    rn = np.concatenate([rel, nbrt], axis=2)  # [B, E, 4096]
    in_maps = []
    for c in range(NCORES):
        sl = slice(c * BC, (c + 1) * BC)
        in_maps.append(
            {
                "rn": np.ascontiguousarray(rn[sl]),
                "selfv": np.ascontiguousarray(self_vectors[sl], dtype=np.float32),
                "side": np.ascontiguousarray(side_embeddings[sl], dtype=np.float32),
                "wmat": np.ascontiguousarray(W, dtype=np.float32),
                "bvec": np.ascontiguousarray(b, dtype=np.float32).reshape(1, D),
                "iden": iden,
            }
        )
    return in_maps


def kernel(self_vectors, neighbor_vectors, neighbor_relations, side_embeddings, W, b,
           _trace=False, _tmpdir=None):
    from concourse import bass_utils

    nc = _get_nc()
    in_maps = _make_in_maps(
        self_vectors, neighbor_vectors, neighbor_relations, side_embeddings, W, b
    )
    res = bass_utils.run_bass_kernel_spmd(
        nc, in_maps, list(range(NCORES)), trace=_trace, tmpdir=_tmpdir
    )
    _CACHE["last_results"] = res
    out = np.concatenate([res.results[c]["out"] for c in range(NCORES)], axis=0)
    return out


# revision 19
# speedup vs baseline: 1.4243x; 1.0038x over previous
"""Trainium2 Bass kernel for nn_Aggregator (GNN message passing).

Computation per (b, e):
  scores[k] = <side[b], rel[b,e,k,:]>          (contract over D=64)
  attn      = softmax_k(scores)
  agg[d]    = sum_k attn[k] * nbr[b,e,k,d]     (contract over K=32)
  out       = relu(cat(self[b,e], agg) @ W + bias)

Sharding: data-parallel over the leading batch dim B=1024 across 8 cores
(128 batches/core). Weights replicated.

Per-core mapping (fully unrolled over 32 "bgroups" of 4 batches), with
ALL tiles on (4b x 32e) partitions so softmax and both contractions are
per-partition free-axis ops on the DVE; the PE only does the final
Linear (transpose + matmul). neighbor_vectors is fed host-permuted to
[b, e, d, k] so the K-contraction is an innermost-axis reduce and every
DMA is fully sequential in DRAM.
"""

import numpy as np

B, E, K, D = 1024, 32, 32, 64
NCORES = 8
BC = B // NCORES  # 128 batches per core
NJ = BC // 4      # 32 bgroups of 4 batches

_CACHE = {}


def _build_nc():
    from contextlib import ExitStack

    import concourse.bass as bass
    import concourse.bacc as bacc
    import concourse.tile as tile
    from concourse import mybir

    f32 = mybir.dt.float32
    Alu = mybir.AluOpType
    Act = mybir.ActivationFunctionType

    # Bacc (not raw Bass): its finalize() legalizes sync waits -- TRN2 allows
    # at most 1 wait per instruction; excess waits split into EventSemaphores.
    nc = bacc.Bacc()

    # rel and nbr^T packed along the last axis: rn[b,e,0:2048] = rel[b,e]
    # (k-major), rn[b,e,2048:4096] = nbr[b,e]^T (d-major) -- one DMA per j.
    rn_h = nc.declare_dram_parameter("rn", [BC, E, 2 * K * D], f32, isOutput=False)
    selfv_h = nc.declare_dram_parameter("selfv", [BC, E, D], f32, isOutput=False)
    side_h = nc.declare_dram_parameter("side", [BC, D], f32, isOutput=False)
    w_h = nc.declare_dram_parameter("wmat", [2 * D, D], f32, isOutput=False)
    b_h = nc.declare_dram_parameter("bvec", [1, D], f32, isOutput=False)
    iden_h = nc.declare_dram_parameter("iden", [128, 128], f32, isOutput=False)
    out_h = nc.declare_dram_parameter("out", [BC, E, D], f32, isOutput=True)

    rn_ap = rn_h[:]
    selfv_ap = selfv_h[:]
    out_ap = out_h[:]

    with tile.TileContext(nc) as tc, ExitStack() as ctx:
        consts = ctx.enter_context(tc.tile_pool(name="consts", bufs=1))
        bigrn = ctx.enter_context(tc.tile_pool(name="bigrn", bufs=5))
        prods = ctx.enter_context(tc.tile_pool(name="prods", bufs=2))
        work = ctx.enter_context(tc.tile_pool(name="work", bufs=4))
        # scores intermediate lives in PSUM: DVE's PSUM port set is separate
        # from its SBUF ports, dodging SBUF read/write contention
        ps_prod = ctx.enter_context(tc.tile_pool(name="ps_prod", bufs=1, space="PSUM"))
        ps_xt = ctx.enter_context(tc.tile_pool(name="ps_xt", bufs=2, space="PSUM"))
        ps_lin = ctx.enter_context(tc.tile_pool(name="ps_lin", bufs=2, space="PSUM"))

        w_sb = consts.tile([128, D], f32)
        nc.sync.dma_start(out=w_sb, in_=w_h[:])
        iden_sb = consts.tile([128, 128], f32)
        nc.sync.dma_start(out=iden_sb, in_=iden_h[:])
        btile = consts.tile([128, D], f32)
        nc.sync.dma_start(
            out=btile,
            in_=bass.AP(tensor=b_h[:].tensor, offset=0, ap=[[0, 128], [1, D]]),
        )
        # side_all[p=(bg,e), j, d] = side[4j + bg, d]; loaded once, one DMA
        # per bg-row block (3-dim AP limit), then copied so the per-j reads
        # depend on a single producer.
        side_all = consts.tile([128, NJ, D], f32)
        for bg in range(4):
            nc.sync.dma_start(
                out=side_all[32 * bg : 32 * bg + 32],
                in_=bass.AP(
                    tensor=side_h[:].tensor,
                    offset=bg * D,
                    ap=[[0, 32], [4 * D, NJ], [1, D]],
                ),
            )

        for j in range(NJ):
            # --- load: one fully sequential DMA, (4b,32e) partitions ---
            rn_sb = bigrn.tile([128, 2 * K * D], f32, tag="rn")
            nc.sync.dma_start(out=rn_sb, in_=rn_ap[4 * j : 4 * j + 4])
            rel_sb = rn_sb[:, 0 : K * D].rearrange("p (k d) -> p k d", k=K)
            nbrt_sb = rn_sb[:, K * D : 2 * K * D].rearrange("p (d k) -> p d k", d=D)

            side4 = side_all[:, j, :]
            # side broadcast over k: [128, K, D] view with a 0-step k dim
            side4_bk = bass.AP(
                tensor=side4.tensor,
                offset=side4.offset,
                ap=[side4.ap[0], [0, K], side4.ap[-1]],
            )

            # --- scores[p, k] = sum_d rel[p,k,d] * side[b(p),d] ---
            prod = ps_prod.tile([128, K, D], f32, tag="prod")
            nc.vector.tensor_mul(out=prod, in0=rel_sb, in1=side4_bk)
            scores = work.tile([128, K], f32, tag="scores")
            nc.vector.tensor_reduce(
                out=scores, in_=prod, axis=mybir.AxisListType.X, op=Alu.add
            )

            # --- softmax over k (free axis; no max-subtraction needed:
            #     |scores| <~ 6*sqrt(64) stays well inside f32 exp range) ---
            escores = work.tile([128, K], f32, tag="escores")
            sums = work.tile([128, 1], f32, tag="sums")
            nc.scalar.activation(
                out=escores, in_=scores, func=Act.Exp, accum_out=sums
            )
            rsums = work.tile([128, 1], f32, tag="rsums")
            nc.vector.reciprocal(out=rsums, in_=sums)
            attn = work.tile([128, K], f32, tag="attn")
            nc.vector.tensor_scalar_mul(out=attn, in0=escores, scalar1=rsums)

            # --- agg[p, d] = sum_k attn[p,k] * nbrt[p,d,k] ---
            attn_bdk = bass.AP(
                tensor=attn.tensor,
                offset=attn.offset,
                ap=[attn.ap[0], [0, D], attn.ap[-1]],
            )
            prod2 = prods.tile([128, D, K], f32, tag="prod2")
            nc.vector.tensor_mul(out=prod2, in0=nbrt_sb, in1=attn_bdk)
            # --- X = cat(self, agg), rows (b, e) ---
            xcat = work.tile([128, 2 * D], f32, tag="xcat")
            nc.sync.dma_start(out=xcat[:, 0:D], in_=selfv_ap[4 * j : 4 * j + 4])
            nc.vector.tensor_reduce(
                out=xcat[:, D : 2 * D], in_=prod2, axis=mybir.AxisListType.X, op=Alu.add
            )

            # --- linear: out = relu(X @ W + b) via X^T on PE ---
            xt_ps = ps_xt.tile([128, 2 * D], f32, tag="xt")
            nc.tensor.transpose(out=xt_ps, in_=xcat, identity=iden_sb)
            xt_sb = work.tile([128, 2 * D], f32, tag="xt_sb")
            nc.scalar.copy(out=xt_sb, in_=xt_ps)
            lin_ps = ps_lin.tile([128, D], f32, tag="lin")
            nc.tensor.matmul(
                out=lin_ps, lhsT=xt_sb, rhs=w_sb, start=True, stop=True
            )
            tmpb = work.tile([128, D], f32, tag="tmpb")
            nc.vector.tensor_add(out=tmpb, in0=lin_ps, in1=btile)
            outb = work.tile([128, D], f32, tag="outb")
            nc.scalar.activation(out=outb, in_=tmpb, func=Act.Relu)
            nc.sync.dma_start(out=out_ap[4 * j : 4 * j + 4], in_=outb)

    nc.finalize()
    return nc


def _get_nc():
    if "nc" not in _CACHE:
        _CACHE["nc"] = _build_nc()
    return _CACHE["nc"]


def _make_in_maps(self_vectors, neighbor_vectors, neighbor_relations, side_embeddings, W, b):
    iden = np.eye(128, dtype=np.float32)
    rel = np.asarray(neighbor_relations, dtype=np.float32).reshape(B, E, K * D)
    nbrt = (
        np.asarray(neighbor_vectors, dtype=np.float32)
        .transpose(0, 1, 3, 2)
        .reshape(B, E, D * K)
    )
    rn = np.concatenate([rel, nbrt], axis=2)  # [B, E, 4096]
    in_maps = []
    for c in range(NCORES):
        sl = slice(c * BC, (c + 1) * BC)
        in_maps.append(
            {
                "rn": np.ascontiguousarray(rn[sl]),
                "selfv": np.ascontiguousarray(self_vectors[sl], dtype=np.float32),
                "side": np.ascontiguousarray(side_embeddings[sl], dtype=np.float32),
                "wmat": np.ascontiguousarray(W, dtype=np.float32),
                "bvec": np.ascontiguousarray(b, dtype=np.float32).reshape(1, D),
                "iden": iden,
            }
        )
    return in_maps


def kernel(self_vectors, neighbor_vectors, neighbor_relations, side_embeddings, W, b,
           _trace=False, _tmpdir=None):
    from concourse import bass_utils

    nc = _get_nc()
    in_maps = _make_in_maps(
        self_vectors, neighbor_vectors, neighbor_relations, side_embeddings, W, b
    )
    res = bass_utils.run_bass_kernel_spmd(
        nc, in_maps, list(range(NCORES)), trace=_trace, tmpdir=_tmpdir
    )
    _CACHE["last_results"] = res
    out = np.concatenate([res.results[c]["out"] for c in range(NCORES)], axis=0)
    return out


# revision 20
# speedup vs baseline: 1.6396x; 1.1512x over previous
"""Trainium2 Bass kernel for nn_Aggregator (GNN message passing).

Computation per (b, e):
  scores[k] = <side[b], rel[b,e,k,:]>          (contract over D=64)
  attn      = softmax_k(scores)
  agg[d]    = sum_k attn[k] * nbr[b,e,k,d]     (contract over K=32)
  out       = relu(cat(self[b,e], agg) @ W + bias)

Sharding: data-parallel over the leading batch dim B=1024 across 8 cores
(128 batches/core); weights replicated.

Per-core mapping, fully unrolled over 32 "bgroups" of 4 batches with all
big tiles on (4b x 32e) partitions:
  - rel and nbr^T arrive host-packed in one tensor (one sequential DMA/j)
  - scores/agg are DVE broadcast-multiply + innermost-axis reduce pairs
  - softmax: ACT exp with fused accum (sum), DVE reciprocal; the 1/sum is
    folded into a single post-scale of the unnormalized aggregate
  - linear: self^T arrives host-transposed; agg^T via one PE transpose;
    out = self@W1 + agg@W2 + 1x128 rank-1 bias, PSUM-accumulated on PE;
    relu on ACT
  - the loop body is software-pipelined: scores(j) is emitted before the
    softmax/agg/linear of j-1, so ACT/PE latency hides under DVE work
"""

import numpy as np

B, E, K, D = 1024, 32, 32, 64
NCORES = 8
BC = B // NCORES  # 128 batches per core
NJ = BC // 4      # 32 bgroups of 4 batches

_CACHE = {}


def _build_nc():
    from contextlib import ExitStack

    import concourse.bass as bass
    import concourse.bacc as bacc
    import concourse.tile as tile
    from concourse import mybir

    f32 = mybir.dt.float32
    Alu = mybir.AluOpType
    Act = mybir.ActivationFunctionType

    # Bacc (not raw Bass): its finalize() legalizes sync waits -- TRN2 allows
    # at most 1 wait per instruction; excess waits split into EventSemaphores.
    nc = bacc.Bacc()

    # rel and nbr^T packed along the last axis: rn[b,e,0:2048] = rel[b,e]
    # (k-major), rn[b,e,2048:4096] = nbr[b,e]^T (d-major) -- one DMA per j.
    rn_h = nc.declare_dram_parameter("rn", [BC, E, 2 * K * D], f32, isOutput=False)
    selft_h = nc.declare_dram_parameter("selft", [D, BC, E], f32, isOutput=False)
    side_h = nc.declare_dram_parameter("side", [BC, D], f32, isOutput=False)
    w_h = nc.declare_dram_parameter("wmat", [2 * D, D], f32, isOutput=False)
    b_h = nc.declare_dram_parameter("bvec", [1, D], f32, isOutput=False)
    ones_h = nc.declare_dram_parameter("ones", [1, 128], f32, isOutput=False)
    iden_h = nc.declare_dram_parameter("iden", [128, 128], f32, isOutput=False)
    out_h = nc.declare_dram_parameter("out", [BC, E, D], f32, isOutput=True)

    rn_ap = rn_h[:]
    selft_ap = selft_h[:]
    out_ap = out_h[:]

    with tile.TileContext(nc) as tc, ExitStack() as ctx:
        consts = ctx.enter_context(tc.tile_pool(name="consts", bufs=1))
        bigrn = ctx.enter_context(tc.tile_pool(name="bigrn", bufs=5))
        prods = ctx.enter_context(tc.tile_pool(name="prods", bufs=2))
        work = ctx.enter_context(tc.tile_pool(name="work", bufs=4))
        # scores intermediate in PSUM: DVE's PSUM ports are separate from its
        # SBUF ports
        ps_prod = ctx.enter_context(tc.tile_pool(name="ps_prod", bufs=1, space="PSUM"))
        ps_at = ctx.enter_context(tc.tile_pool(name="ps_at", bufs=2, space="PSUM"))
        ps_lin = ctx.enter_context(tc.tile_pool(name="ps_lin", bufs=2, space="PSUM"))

        w1_sb = consts.tile([D, D], f32)
        nc.sync.dma_start(out=w1_sb, in_=w_h[:][0:D])
        w2_sb = consts.tile([D, D], f32)
        nc.sync.dma_start(out=w2_sb, in_=w_h[:][D : 2 * D])
        bvec_sb = consts.tile([1, D], f32)
        nc.sync.dma_start(out=bvec_sb, in_=b_h[:])
        ones_sb = consts.tile([1, 128], f32)
        nc.sync.dma_start(out=ones_sb, in_=ones_h[:])
        iden_sb = consts.tile([128, 128], f32)
        nc.sync.dma_start(out=iden_sb, in_=iden_h[:])
        # side_all[p=(bg,e), j, d] = side[4j + bg, d]; loaded once, one DMA
        # per bg-row block (3-dim AP limit).
        side_all = consts.tile([128, NJ, D], f32)
        for bg in range(4):
            nc.sync.dma_start(
                out=side_all[32 * bg : 32 * bg + 32],
                in_=bass.AP(
                    tensor=side_h[:].tensor,
                    offset=bg * D,
                    ap=[[0, 32], [4 * D, NJ], [1, D]],
                ),
            )

        # pipeline state carried from stage A (scores) to stage B (rest)
        st = {}

        def stage_a(j):
            rn_sb = bigrn.tile([128, 2 * K * D], f32, tag="rn")
            nc.sync.dma_start(out=rn_sb, in_=rn_ap[4 * j : 4 * j + 4])
            rel_sb = rn_sb[:, 0 : K * D].rearrange("p (k d) -> p k d", k=K)

            side4 = side_all[:, j, :]
            side4_bk = bass.AP(
                tensor=side4.tensor,
                offset=side4.offset,
                ap=[side4.ap[0], [0, K], side4.ap[-1]],
            )

            # scores[p, k] = sum_d rel[p,k,d] * side[b(p),d]
            prod = ps_prod.tile([128, K, D], f32, tag="prod")
            nc.vector.tensor_mul(out=prod, in0=rel_sb, in1=side4_bk)
            scores = work.tile([128, K], f32, tag="scores")
            nc.vector.tensor_reduce(
                out=scores, in_=prod, axis=mybir.AxisListType.X, op=Alu.add
            )
            # exp with fused row-sum (no max-subtraction: |scores| <~ 6*sqrt(64)
            # stays well inside the f32 exp range)
            escores = work.tile([128, K], f32, tag="escores")
            sums = work.tile([128, 1], f32, tag="sums")
            nc.scalar.activation(
                out=escores, in_=scores, func=Act.Exp, accum_out=sums
            )
            st[j] = (rn_sb, escores, sums)

        def stage_b(j):
            rn_sb, escores, sums = st.pop(j)
            nbrt_sb = rn_sb[:, K * D : 2 * K * D].rearrange("p (d k) -> p d k", d=D)

            rsums = work.tile([128, 1], f32, tag="rsums")
            nc.vector.reciprocal(out=rsums, in_=sums)

            # agg_u[p, d] = sum_k escores[p,k] * nbrt[p,d,k]; then scale by 1/sum
            esc_bdk = bass.AP(
                tensor=escores.tensor,
                offset=escores.offset,
                ap=[escores.ap[0], [0, D], escores.ap[-1]],
            )
            prod2 = prods.tile([128, D, K], f32, tag="prod2")
            nc.vector.tensor_mul(out=prod2, in0=nbrt_sb, in1=esc_bdk)
            agg_u = work.tile([128, D], f32, tag="agg_u")
            nc.vector.tensor_reduce(
                out=agg_u, in_=prod2, axis=mybir.AxisListType.X, op=Alu.add
            )
            agg = work.tile([128, D], f32, tag="agg")
            nc.vector.tensor_scalar_mul(out=agg, in0=agg_u, scalar1=rsums)

            # linear: lin = self@W1 + agg@W2 + ones^T b (PSUM-accumulated)
            selft_sb = work.tile([D, 128], f32, tag="selft_sb")
            nc.sync.dma_start(out=selft_sb, in_=selft_ap[:, 4 * j : 4 * j + 4, :])
            at_ps = ps_at.tile([D, 128], f32, tag="at")
            nc.tensor.transpose(out=at_ps, in_=agg, identity=iden_sb)
            at_sb = work.tile([D, 128], f32, tag="at_sb")
            nc.scalar.copy(out=at_sb, in_=at_ps)
            lin_ps = ps_lin.tile([128, D], f32, tag="lin")
            nc.tensor.matmul(
                out=lin_ps, lhsT=selft_sb, rhs=w1_sb, start=True, stop=False
            )
            nc.tensor.matmul(
                out=lin_ps, lhsT=at_sb, rhs=w2_sb, start=False, stop=False
            )
            nc.tensor.matmul(
                out=lin_ps, lhsT=ones_sb, rhs=bvec_sb, start=False, stop=True
            )
            outb = work.tile([128, D], f32, tag="outb")
            nc.scalar.activation(out=outb, in_=lin_ps, func=Act.Relu)
            nc.sync.dma_start(out=out_ap[4 * j : 4 * j + 4], in_=outb)

        for j in range(NJ + 1):
            if j < NJ:
                stage_a(j)
            if j >= 1:
                stage_b(j - 1)

    nc.finalize()
    return nc


def _get_nc():
    if "nc" not in _CACHE:
        _CACHE["nc"] = _build_nc()
    return _CACHE["nc"]


def _make_in_maps(self_vectors, neighbor_vectors, neighbor_relations, side_embeddings, W, b):
    iden = np.eye(128, dtype=np.float32)
    ones = np.ones((1, 128), dtype=np.float32)
    rel = np.asarray(neighbor_relations, dtype=np.float32).reshape(B, E, K * D)
    nbrt = (
        np.asarray(neighbor_vectors, dtype=np.float32)
        .transpose(0, 1, 3, 2)
        .reshape(B, E, D * K)
    )
    rn = np.concatenate([rel, nbrt], axis=2)  # [B, E, 4096]
    sv = np.asarray(self_vectors, dtype=np.float32)
    in_maps = []
    for c in range(NCORES):
        sl = slice(c * BC, (c + 1) * BC)
        in_maps.append(
            {
                "rn": np.ascontiguousarray(rn[sl]),
                "selft": np.ascontiguousarray(sv[sl].transpose(2, 0, 1)),
                "side": np.ascontiguousarray(side_embeddings[sl], dtype=np.float32),
                "wmat": np.ascontiguousarray(W, dtype=np.float32),
                "bvec": np.ascontiguousarray(b, dtype=np.float32).reshape(1, D),
                "ones": ones,
                "iden": iden,
            }
        )
    return in_maps


def kernel(self_vectors, neighbor_vectors, neighbor_relations, side_embeddings, W, b,
           _trace=False, _tmpdir=None):
    from concourse import bass_utils

    nc = _get_nc()
    in_maps = _make_in_maps(
        self_vectors, neighbor_vectors, neighbor_relations, side_embeddings, W, b
    )
    res = bass_utils.run_bass_kernel_spmd(
        nc, in_maps, list(range(NCORES)), trace=_trace, tmpdir=_tmpdir
    )
    _CACHE["last_results"] = res
    out = np.concatenate([res.results[c]["out"] for c in range(NCORES)], axis=0)
    return out


# revision 23
# speedup vs baseline: 1.6589x; 1.0118x over previous
"""Trainium2 Bass kernel for nn_Aggregator (GNN message passing).

Computation per (b, e):
  scores[k] = <side[b], rel[b,e,k,:]>          (contract over D=64)
  attn      = softmax_k(scores)
  agg[d]    = sum_k attn[k] * nbr[b,e,k,d]     (contract over K=32)
  out       = relu(cat(self[b,e], agg) @ W + bias)

Sharding: data-parallel over the leading batch dim B=1024 across 8 cores
(128 batches/core); weights replicated.

Per-core mapping, fully unrolled over 32 "bgroups" of 4 batches with all
big tiles on (4b x 32e) partitions:
  - rel and nbr^T arrive host-packed in one tensor (one sequential DMA/j)
  - scores/agg are DVE broadcast-multiply + innermost-axis reduce pairs
  - softmax: ACT exp with fused accum (sum), DVE reciprocal; the 1/sum is
    folded into a single post-scale of the unnormalized aggregate
  - linear: self^T arrives host-transposed; agg^T via one PE transpose;
    out = self@W1 + agg@W2 + 1x128 rank-1 bias, PSUM-accumulated on PE;
    relu on ACT
  - the loop body is software-pipelined: scores(j) is emitted before the
    softmax/agg/linear of j-1, so ACT/PE latency hides under DVE work
"""

import numpy as np

B, E, K, D = 1024, 32, 32, 64
NCORES = 8
BC = B // NCORES  # 128 batches per core
NJ = BC // 4      # 32 bgroups of 4 batches

_CACHE = {}


def _build_nc():
    from contextlib import ExitStack

    import concourse.bass as bass
    import concourse.bacc as bacc
    import concourse.tile as tile
    from concourse import mybir

    f32 = mybir.dt.float32
    Alu = mybir.AluOpType
    Act = mybir.ActivationFunctionType

    # Bacc (not raw Bass): its finalize() legalizes sync waits -- TRN2 allows
    # at most 1 wait per instruction; excess waits split into EventSemaphores.
    nc = bacc.Bacc()

    # rel and nbr^T packed along the last axis: rn[b,e,0:2048] = rel[b,e]
    # (k-major), rn[b,e,2048:4096] = nbr[b,e]^T (d-major) -- one DMA per j.
    rn_h = nc.declare_dram_parameter("rn", [BC, E, 2 * K * D], f32, isOutput=False)
    selft_h = nc.declare_dram_parameter("selft", [D, BC, E], f32, isOutput=False)
    side_h = nc.declare_dram_parameter("side", [BC, D], f32, isOutput=False)
    w_h = nc.declare_dram_parameter("wmat", [2 * D, D], f32, isOutput=False)
    b_h = nc.declare_dram_parameter("bvec", [1, D], f32, isOutput=False)
    ones_h = nc.declare_dram_parameter("ones", [1, 128], f32, isOutput=False)
    iden_h = nc.declare_dram_parameter("iden", [128, 128], f32, isOutput=False)
    out_h = nc.declare_dram_parameter("out", [BC, E, D], f32, isOutput=True)

    rn_ap = rn_h[:]
    selft_ap = selft_h[:]
    out_ap = out_h[:]

    with tile.TileContext(nc) as tc, ExitStack() as ctx:
        consts = ctx.enter_context(tc.tile_pool(name="consts", bufs=1))
        bigrn = ctx.enter_context(tc.tile_pool(name="bigrn", bufs=5))
        prods = ctx.enter_context(tc.tile_pool(name="prods", bufs=2))
        work = ctx.enter_context(tc.tile_pool(name="work", bufs=4))
        # scores intermediate in PSUM: DVE's PSUM ports are separate from its
        # SBUF ports
        ps_prod = ctx.enter_context(tc.tile_pool(name="ps_prod", bufs=1, space="PSUM"))
        ps_at = ctx.enter_context(tc.tile_pool(name="ps_at", bufs=2, space="PSUM"))
        ps_lin = ctx.enter_context(tc.tile_pool(name="ps_lin", bufs=2, space="PSUM"))

        w1_sb = consts.tile([D, D], f32)
        nc.sync.dma_start(out=w1_sb, in_=w_h[:][0:D])
        w2_sb = consts.tile([D, D], f32)
        nc.sync.dma_start(out=w2_sb, in_=w_h[:][D : 2 * D])
        bvec_sb = consts.tile([1, D], f32)
        nc.sync.dma_start(out=bvec_sb, in_=b_h[:])
        ones_sb = consts.tile([1, 128], f32)
        nc.sync.dma_start(out=ones_sb, in_=ones_h[:])
        iden_sb = consts.tile([128, 128], f32)
        nc.sync.dma_start(out=iden_sb, in_=iden_h[:])
        # side_all[p=(bg,e), j, d] = side[4j + bg, d]; loaded once, one DMA
        # per bg-row block (3-dim AP limit).
        side_all = consts.tile([128, NJ, D], f32)
        for bg in range(4):
            nc.sync.dma_start(
                out=side_all[32 * bg : 32 * bg + 32],
                in_=bass.AP(
                    tensor=side_h[:].tensor,
                    offset=bg * D,
                    ap=[[0, 32], [4 * D, NJ], [1, D]],
                ),
            )

        # pipeline state carried from stage A (scores) to stage B (rest)
        st = {}

        def stage_a(j):
            rn_sb = bigrn.tile([128, 2 * K * D], f32, tag="rn")
            nc.sync.dma_start(out=rn_sb, in_=rn_ap[4 * j : 4 * j + 4])
            rel_sb = rn_sb[:, 0 : K * D].rearrange("p (k d) -> p k d", k=K)

            side4 = side_all[:, j, :]
            side4_bk = bass.AP(
                tensor=side4.tensor,
                offset=side4.offset,
                ap=[side4.ap[0], [0, K], side4.ap[-1]],
            )

            # scores[p, k] = sum_d rel[p,k,d] * side[b(p),d]
            prod = ps_prod.tile([128, K, D], f32, tag="prod")
            nc.vector.tensor_mul(out=prod, in0=rel_sb, in1=side4_bk)
            scores = work.tile([128, K], f32, tag="scores")
            nc.vector.tensor_reduce(
                out=scores, in_=prod, axis=mybir.AxisListType.X, op=Alu.add
            )
            # exp on ACT (no max-subtraction: |scores| <~ 6*sqrt(64) stays well
            # inside the f32 exp range); row-sum on DVE so the downstream
            # reciprocal never waits on a cross-engine accumulator drain
            escores = work.tile([128, K], f32, tag="escores")
            nc.scalar.activation(out=escores, in_=scores, func=Act.Exp)
            st[j] = (rn_sb, escores)

        def stage_b(j):
            rn_sb, escores = st.pop(j)
            nbrt_sb = rn_sb[:, K * D : 2 * K * D].rearrange("p (d k) -> p d k", d=D)

            sums = work.tile([128, 1], f32, tag="sums")
            nc.vector.tensor_reduce(
                out=sums, in_=escores, axis=mybir.AxisListType.X, op=Alu.add
            )
            rsums = work.tile([128, 1], f32, tag="rsums")
            nc.vector.reciprocal(out=rsums, in_=sums)

            # agg_u[p, d] = sum_k escores[p,k] * nbrt[p,d,k]; then scale by 1/sum
            esc_bdk = bass.AP(
                tensor=escores.tensor,
                offset=escores.offset,
                ap=[escores.ap[0], [0, D], escores.ap[-1]],
            )
            prod2 = prods.tile([128, D, K], f32, tag="prod2")
            nc.vector.tensor_mul(out=prod2, in0=nbrt_sb, in1=esc_bdk)
            agg_u = work.tile([128, D], f32, tag="agg_u")
            nc.vector.tensor_reduce(
                out=agg_u, in_=prod2, axis=mybir.AxisListType.X, op=Alu.add
            )
            agg = work.tile([128, D], f32, tag="agg")
            nc.vector.tensor_scalar_mul(out=agg, in0=agg_u, scalar1=rsums)

            # linear: lin = self@W1 + agg@W2 + ones^T b (PSUM-accumulated)
            selft_sb = work.tile([D, 128], f32, tag="selft_sb")
            nc.sync.dma_start(out=selft_sb, in_=selft_ap[:, 4 * j : 4 * j + 4, :])
            at_ps = ps_at.tile([D, 128], f32, tag="at")
            nc.tensor.transpose(out=at_ps, in_=agg, identity=iden_sb)
            at_sb = work.tile([D, 128], f32, tag="at_sb")
            nc.scalar.copy(out=at_sb, in_=at_ps)
            lin_ps = ps_lin.tile([128, D], f32, tag="lin")
            nc.tensor.matmul(
                out=lin_ps, lhsT=selft_sb, rhs=w1_sb, start=True, stop=False
            )
            nc.tensor.matmul(
                out=lin_ps, lhsT=at_sb, rhs=w2_sb, start=False, stop=False
            )
            nc.tensor.matmul(
                out=lin_ps, lhsT=ones_sb, rhs=bvec_sb, start=False, stop=True
            )
            outb = work.tile([128, D], f32, tag="outb")
            nc.scalar.activation(out=outb, in_=lin_ps, func=Act.Relu)
            nc.sync.dma_start(out=out_ap[4 * j : 4 * j + 4], in_=outb)

        for j in range(NJ + 1):
            if j < NJ:
                stage_a(j)
            if j >= 1:
                stage_b(j - 1)

    nc.finalize()
    return nc


def _get_nc():
    if "nc" not in _CACHE:
        _CACHE["nc"] = _build_nc()
    return _CACHE["nc"]


def _make_in_maps(self_vectors, neighbor_vectors, neighbor_relations, side_embeddings, W, b):
    iden = np.eye(128, dtype=np.float32)
    ones = np.ones((1, 128), dtype=np.float32)
    rel = np.asarray(neighbor_relations, dtype=np.float32).reshape(B, E, K * D)
    nbrt = (
        np.asarray(neighbor_vectors, dtype=np.float32)
        .transpose(0, 1, 3, 2)
        .reshape(B, E, D * K)
    )
    rn = np.concatenate([rel, nbrt], axis=2)  # [B, E, 4096]
    sv = np.asarray(self_vectors, dtype=np.float32)
    in_maps = []
    for c in range(NCORES):
        sl = slice(c * BC, (c + 1) * BC)
        in_maps.append(
            {
                "rn": np.ascontiguousarray(rn[sl]),
                "selft": np.ascontiguousarray(sv[sl].transpose(2, 0, 1)),
                "side": np.ascontiguousarray(side_embeddings[sl], dtype=np.float32),
                "wmat": np.ascontiguousarray(W, dtype=np.float32),
                "bvec": np.ascontiguousarray(b, dtype=np.float32).reshape(1, D),
                "ones": ones,
                "iden": iden,
            }
        )
    return in_maps


def kernel(self_vectors, neighbor_vectors, neighbor_relations, side_embeddings, W, b,
           _trace=False, _tmpdir=None):
    from concourse import bass_utils

    nc = _get_nc()
    in_maps = _make_in_maps(
        self_vectors, neighbor_vectors, neighbor_relations, side_embeddings, W, b
    )
    res = bass_utils.run_bass_kernel_spmd(
        nc, in_maps, list(range(NCORES)), trace=_trace, tmpdir=_tmpdir
    )
    _CACHE["last_results"] = res
    out = np.concatenate([res.results[c]["out"] for c in range(NCORES)], axis=0)
    return out
